# revision 1
# baseline (speedup 1.0000x reference)
"""Domain-specific BatchNorm (8 domains) on 8 Trainium2 NeuronCores.

Strategy (data-parallel over rows, per the spec sharding hint):
  - Shard x/y row-wise across 8 cores (32768 rows each). The host also ships
    a bf16 copy of x plus tiny bf16 one-hot encodings of y (exact 0/1).
  - Pass 1 (stats, ~55us, TensorE-bound): stream the bf16 x in 512-row
    "quad" DMAs (16MB/core); square on DVE/ScalarE; accumulate per-domain
    s1 = onehot^T @ x and s2 = onehot^T @ x^2 on the TensorEngine into PSUM
    (pairs of 128-row tiles share one matmul via a 40-wide one-hot with the
    high half at partition offset 32 - PSUM reads must start at 0 mod 32).
    Counts accumulate on DVE with one final matmul.
  - AllReduce the [8, 513] partials (s1 | s2 | count) across the 8 cores
    (~10us, overlapped with pass-2 x prefetch).
  - Per-domain affine A = gamma*rsqrt(var+eps), B = beta - mean*A (identity
    for domains with <2 samples), split A into exact bf16 hi+lo parts.
  - Pass 2 (normalize, ~175us, DMA/DVE co-bound): stream f32 x once
    (30MB; 8 quads keep their pass-1 bf16 copy resident and stream only the
    bf16 residual xlo = x - bf16(x), reconstructing x = xb + xlo on DVE to
    ~1.5e-5); per 128-row tile gather A[y], B[y] rows via a transposed-
    one-hot matmul (A_hi+A_lo accumulated in PSUM -> fp32-accurate; B needs
    only hi since beta=0 makes |B|~1e-2), ScalarE copies PSUM->SBUF,
    VectorE computes out = x*A[y] + B[y] quad-wide, stream out (32MB).

Total HBM traffic ~78MB/core; TimelineSim cost model: ~258us/core.
bf16 only ever touches (a) stats inputs, where rounding error averages out
across ~32k samples per domain, and (b) exact 0/1 one-hot weights; the A
table goes through an exact hi+lo bf16 split, so the output matches the
fp32 reference to ~6e-5 relative (~3e-4 max-abs at scale ~5).
"""

import sys

if "/opt/trn_rl_repo" not in sys.path:
    sys.path.insert(0, "/opt/trn_rl_repo")

import numpy as np
import ml_dtypes

import concourse.bass as bass
import concourse.tile as tile
from concourse import bacc, mybir
from concourse import bass_utils

F32 = mybir.dt.float32
BF16 = mybir.dt.bfloat16
AF = mybir.ActivationFunctionType
ALU = mybir.AluOpType

N = 262144
F = 256
D = 8
CORES = 8
NR = N // CORES          # rows per core
EPS = 1e-5
MW = 40                  # one-hot stationary width per pair (high half at +32)
OT_CHUNK_TILES = 8      # tiles covered per resident oT chunk


def build_program(nr=NR, num_devices=CORES, use_collective=True, r_quads=8):
    """Build (and compile) the SPMD bass program for `nr` rows per core."""
    quads = nr // 512
    assert nr % 512 == 0
    pairs_per_quad = 2
    ot_chunks = max(1, (nr // 128) // OT_CHUNK_TILES)
    n_res = min(r_quads, quads)
    # spread the xb-resident quads so pass-2 DMA load stays uniform
    res_qs = sorted({int((i + 0.5) * quads / n_res) for i in range(n_res)}) if n_res else []
    res_index = {q: i for i, q in enumerate(res_qs)}

    nc = bacc.Bacc(
        "TRN2",
        target_bir_lowering=False,
        debug=False,
        enable_asserts=False,
        num_devices=num_devices,
    )

    x_d = nc.dram_tensor("x", [nr, F], F32, kind="ExternalInput")
    xb_d = nc.dram_tensor("xb", [nr, F], BF16, kind="ExternalInput")
    xlo_d = nc.dram_tensor("xlo", [nr, F], BF16, kind="ExternalInput")
    oh16_d = nc.dram_tensor(
        "oh16", [128, (nr // 256) * MW], BF16, kind="ExternalInput"
    )
    oT_d = nc.dram_tensor("oT", [D, nr], BF16, kind="ExternalInput")
    gamma_d = nc.dram_tensor("gamma", [D, F], F32, kind="ExternalInput")
    beta_d = nc.dram_tensor("beta", [D, F], F32, kind="ExternalInput")
    out_d = nc.dram_tensor("out", [nr, F], F32, kind="ExternalOutput")

    def quad_ap(dram, q):
        # [512 rows, F] -> [128 partitions, 4, 256]: [:, j, :] = row q*512+j*128+p
        return dram[q * 512 : (q + 1) * 512, :].rearrange(
            "(four p) f -> p four f", four=4
        )

    def as4d(sbuf_ap):
        return sbuf_ap.rearrange("p (four f) -> p four f", four=4)

    with tile.TileContext(nc) as tc:
        with (
            tc.tile_pool(name="resident", bufs=1) as resident,
            tc.tile_pool(name="otc", bufs=2) as ot_pool,
            tc.tile_pool(name="xbq", bufs=6) as xb_pool,
            tc.tile_pool(name="xbres", bufs=1) as xbres_pool,
            tc.tile_pool(name="xlo", bufs=4) as xlo_pool,
            tc.tile_pool(name="xrec", bufs=3) as xrec_pool,
            tc.tile_pool(name="xstream", bufs=12) as xstream_pool,
            tc.tile_pool(name="xx", bufs=4) as xx_pool,
            tc.tile_pool(name="ab", bufs=4) as ab_pool,
            tc.tile_pool(name="outp", bufs=4) as out_pool,
            tc.tile_pool(name="smalls", bufs=1) as smalls,
            tc.tile_pool(name="dram", bufs=1, space="DRAM") as dram,
        ):
            # ---- resident inputs ----
            n_oh = 1
            oh_cols = (nr // 256) * MW // n_oh
            oh16_tiles = []
            for i in range(n_oh):
                ot_ = resident.tile([128, oh_cols], BF16, name=f"oh16_{i}")
                nc.sync.dma_start(
                    ot_[:], oh16_d[:, i * oh_cols : (i + 1) * oh_cols]
                )
                oh16_tiles.append(ot_)

            def oh16_slice(g):
                i = (g * MW) // oh_cols
                off = g * MW - i * oh_cols
                return oh16_tiles[i][:, off : off + MW]
            gsb = smalls.tile([D, F], F32)
            nc.sync.dma_start(gsb[:], gamma_d[:, :])
            bsb = smalls.tile([D, F], F32)
            nc.sync.dma_start(bsb[:], beta_d[:, :])
            xbres = [
                xbres_pool.tile([128, 1024], BF16, name=f"xbres{i}")
                for i in range(len(res_qs))
            ]
            ones_col = smalls.tile([128, 1], F32)
            nc.vector.memset(ones_col[:], 1.0)
            oh_acc = smalls.tile([128, MW], F32)
            nc.vector.memset(oh_acc[:], 0.0)

            # ---- pass 1: per-domain partial sums ----
            stats = smalls.tile([D, 513], F32)
            with tc.tile_pool(
                name="psum_stats", bufs=1, space="PSUM"
            ) as psum_stats:
                psum_A = psum_stats.tile([MW, 512], F32)
                psum_B = psum_stats.tile([MW, 512], F32)
                psum_c = psum_stats.tile([MW, 1], F32)

                for q in range(quads):
                    if q in res_index:
                        xbq = xbres[res_index[q]]
                    else:
                        xbq = xb_pool.tile([128, 1024], BF16)
                    nc.sync.dma_start(as4d(xbq[:]), quad_ap(xb_d, q))
                    xx = xx_pool.tile([128, 1024], BF16)
                    if q % 2 == 0:
                        nc.scalar.activation(xx[:], xbq[:], AF.Square)
                    else:
                        nc.vector.tensor_mul(xx[:], xbq[:], xbq[:])
                    # accumulate one-hot columns for counts on DVE (PE slack)
                    nc.vector.tensor_add(
                        oh_acc[:], oh_acc[:], oh16_slice(q * 2)
                    )
                    nc.vector.tensor_add(
                        oh_acc[:], oh_acc[:], oh16_slice(q * 2 + 1)
                    )
                    for hp in range(pairs_per_quad):
                        g = q * 2 + hp
                        lhsT = oh16_slice(g)
                        first = g == 0
                        last = g == 2 * quads - 1
                        nc.tensor.matmul(
                            psum_A[:],
                            lhsT,
                            xbq[:, hp * 512 : (hp + 1) * 512],
                            start=first,
                            stop=last,
                        )
                        nc.tensor.matmul(
                            psum_B[:],
                            lhsT,
                            xx[:, hp * 512 : (hp + 1) * 512],
                            start=first,
                            stop=last,
                        )


                nc.tensor.matmul(
                    psum_c[:], oh_acc[:], ones_col[:], start=True, stop=True
                )

                # fold low/high quadrants -> [8, 513] partial stats
                # (PSUM has a single DVE read port: copy one quadrant out first)
                nc.vector.tensor_copy(stats[:, 0:256], psum_A[0:8, 0:256])
                nc.vector.tensor_add(
                    stats[:, 0:256], stats[:, 0:256], psum_A[32:40, 256:512]
                )
                nc.vector.tensor_copy(stats[:, 256:512], psum_B[0:8, 0:256])
                nc.vector.tensor_add(
                    stats[:, 256:512], stats[:, 256:512], psum_B[32:40, 256:512]
                )
                nc.vector.tensor_copy(stats[:, 512:513], psum_c[0:8, :])
                nc.vector.tensor_add(
                    stats[:, 512:513], stats[:, 512:513], psum_c[32:40, :]
                )

            # ---- all-reduce partials across cores ----
            gstats = smalls.tile([D, 513], F32)
            if use_collective:
                cc_in = dram.tile([D, 513], F32)
                cc_space = "Shared" if num_devices > 4 else "Local"
                cc_out = dram.tile([D, 513], F32, addr_space=cc_space)
                nc.sync.dma_start(cc_in[:], stats[:])
                nc.gpsimd.collective_compute(
                    "AllReduce",
                    ALU.add,
                    replica_groups=[list(range(num_devices))],
                    ins=[cc_in.opt()],
                    outs=[cc_out.opt()],
                )
                nc.sync.dma_start(gstats[:], cc_out[:])
            else:
                nc.vector.tensor_copy(gstats[:], stats[:])

            # ---- per-domain affine coefficients ----
            cnt = smalls.tile([D, 1], F32)
            nc.vector.tensor_scalar_max(cnt[:], gstats[:, 512:513], 1.0)
            rc = smalls.tile([D, 1], F32)
            nc.vector.reciprocal(rc[:], cnt[:])
            mean = smalls.tile([D, F], F32)
            nc.vector.tensor_scalar_mul(mean[:], gstats[:, 0:256], rc[:])
            var = smalls.tile([D, F], F32)
            nc.vector.tensor_scalar_mul(var[:], gstats[:, 256:512], rc[:])
            m2 = smalls.tile([D, F], F32)
            nc.vector.tensor_mul(m2[:], mean[:], mean[:])
            nc.vector.tensor_sub(var[:], var[:], m2[:])
            # fp roundoff can leave var a hair negative when true var == 0
            nc.vector.tensor_scalar_max(var[:], var[:], 0.0)
            eps_ap = smalls.tile([D, 1], F32)
            nc.vector.memset(eps_ap[:], EPS)
            std = smalls.tile([D, F], F32)
            nc.scalar.activation(std[:], var[:], AF.Sqrt, bias=eps_ap[:])
            istd = smalls.tile([D, F], F32)
            nc.vector.reciprocal(istd[:], std[:])
            # use_bn mask: 1.0 where count > 1 else 0.0
            mask = smalls.tile([D, 1], F32)
            nc.vector.tensor_scalar(
                mask[:], gstats[:, 512:513], 1.0, None, op0=ALU.is_gt
            )

            ab_f = smalls.tile([D, 512], F32)
            a_f = ab_f[:, 0:256]
            b_f = ab_f[:, 256:512]
            # G = gamma * istd;  A = (G-1)*mask + 1
            nc.vector.tensor_mul(a_f, gsb[:], istd[:])
            # B = (beta - mean*G) * mask   (compute before A is remapped)
            nc.vector.tensor_mul(b_f, mean[:], a_f)
            nc.vector.tensor_sub(b_f, bsb[:], b_f)
            nc.vector.tensor_scalar_mul(b_f, b_f, mask[:])
            nc.vector.tensor_scalar_add(a_f, a_f, -1.0)
            nc.vector.tensor_scalar_mul(a_f, a_f, mask[:])
            nc.vector.tensor_scalar_add(a_f, a_f, 1.0)

            # bf16 hi/lo split; the lo correction is only needed for A:
            # with the spec's beta=0, |B| = |mean*G| ~ 1e-2, so bf16 B is
            # already ~4e-5-absolute accurate.
            ab_hi = smalls.tile([D, 512], BF16)
            nc.vector.tensor_copy(ab_hi[:], ab_f[:])
            hi_f = smalls.tile([D, 256], F32)
            nc.vector.tensor_copy(hi_f[:], ab_hi[:, 0:256])
            lo_f = smalls.tile([D, 256], F32)
            nc.vector.tensor_sub(lo_f[:], a_f, hi_f[:])
            a_lo = smalls.tile([D, 256], BF16)
            nc.vector.tensor_copy(a_lo[:], lo_f[:])

            # ---- pass 2: normalize ----
            psum_ab_pool = tc.alloc_tile_pool(name="psum_ab", bufs=2, space="PSUM")
            ot_tiles_per_chunk = (nr // 128) // ot_chunks

            def get_ot_chunk(c, cache={}):
                if c not in cache:
                    otc = ot_pool.tile([D, ot_tiles_per_chunk * 128], BF16)
                    nc.sync.dma_start(
                        otc[:],
                        oT_d[
                            :,
                            c * ot_tiles_per_chunk * 128 : (c + 1)
                            * ot_tiles_per_chunk
                            * 128,
                        ],
                    )
                    cache[c] = otc
                return cache[c]

            for q in range(quads):
                if q in res_index:
                    xloq = xlo_pool.tile([128, 1024], BF16)
                    nc.sync.dma_start(as4d(xloq[:]), quad_ap(xlo_d, q))
                    xin2 = xrec_pool.tile([128, 1024], F32, name="xr", tag="xr")
                    nc.vector.tensor_add(
                        xin2[:], xbres[res_index[q]][:], xloq[:]
                    )
                else:
                    xin2 = xstream_pool.tile([128, 1024], F32, name="xs2", tag="xs")
                    nc.sync.dma_start(as4d(xin2[:]), quad_ap(x_d, q))
                outp = out_pool.tile([128, 1024], F32)
                ab = ab_pool.tile([128, 2048], F32)
                # one [128, 2048] PSUM (4 banks) holds [A|B] for all 4 tiles
                psum_ab = psum_ab_pool.tile([128, 2048], F32)
                for j in range(4):
                    t = q * 4 + j
                    c = t // ot_tiles_per_chunk
                    otc = get_ot_chunk(c)
                    r = t % ot_tiles_per_chunk
                    lhsT = otc[:, r * 128 : (r + 1) * 128]
                    nc.tensor.matmul(
                        psum_ab[:, j * 512 : (j + 1) * 512],
                        lhsT,
                        ab_hi[:],
                        start=True,
                        stop=True,
                        skip_group_check=True,
                    )
                    nc.tensor.matmul(
                        psum_ab[:, j * 512 : j * 512 + 256],
                        lhsT,
                        a_lo[:],
                        start=False,
                        stop=True,
                        skip_group_check=True,
                    )
                nc.scalar.activation(ab[:], psum_ab[:], AF.Copy)
                # out = x * A + B quad-wide (3D strided views over ab)
                xsl = xin2[:].rearrange("p (t f) -> p t f", t=4)
                osl = outp[:].rearrange("p (t f) -> p t f", t=4)
                a_view = ab[:].rearrange("p (t f) -> p t f", t=4)[:, :, 0:256]
                b_view = ab[:].rearrange("p (t f) -> p t f", t=4)[:, :, 256:512]
                nc.vector.tensor_mul(osl, xsl, a_view)
                nc.vector.tensor_add(osl, osl, b_view)
                nc.scalar.dma_start(quad_ap(out_d, q), as4d(outp[:]))
            psum_ab_pool.release()

    nc.compile()
    return nc


def host_prep(x, y, gamma, beta, nr=NR, num_devices=CORES):
    """Shard + encode inputs per core."""
    x = np.ascontiguousarray(np.asarray(x, dtype=np.float32))
    y = np.asarray(y, dtype=np.int32)
    gamma = np.ascontiguousarray(np.asarray(gamma, dtype=np.float32))
    beta = np.ascontiguousarray(np.asarray(beta, dtype=np.float32))
    dom = np.arange(D, dtype=np.int32)
    in_maps = []
    for c in range(num_devices):
        ys = y[c * nr : (c + 1) * nr]
        pairs = nr // 256
        ohw = np.zeros((pairs, 128, MW), dtype=ml_dtypes.bfloat16)
        yp = ys.reshape(pairs, 2, 128)
        ohw[:, :, 0:8] = yp[:, 0, :, None] == dom
        ohw[:, :, 32:40] = yp[:, 1, :, None] == dom
        oh16 = np.ascontiguousarray(ohw.transpose(1, 0, 2).reshape(128, -1))
        oT = np.ascontiguousarray((ys[None, :] == dom[:, None])).astype(
            ml_dtypes.bfloat16
        )
        xs = x[c * nr : (c + 1) * nr]
        xbs = xs.astype(ml_dtypes.bfloat16)
        xlos = (xs - xbs.astype(np.float32)).astype(ml_dtypes.bfloat16)
        in_maps.append(
            {
                "x": xs,
                "xb": xbs,
                "xlo": xlos,
                "oh16": oh16,
                "oT": oT,
                "gamma": gamma,
                "beta": beta,
            }
        )
    return in_maps


_CACHE = {}


def _get_program():
    if "nc" not in _CACHE:
        _CACHE["nc"] = build_program()
    return _CACHE["nc"]


def kernel(x, y, gamma, beta):
    nc = _get_program()
    in_maps = host_prep(x, y, gamma, beta)
    res = bass_utils.run_bass_kernel_spmd(nc, in_maps, core_ids=list(range(CORES)))
    out = np.empty((N, F), dtype=np.float32)
    for c in range(CORES):
        out[c * NR : (c + 1) * NR] = res.results[c]["out"]
    return out



# revision 3
# speedup vs baseline: 1.1550x; 1.1550x over previous
"""Domain-specific BatchNorm (8 domains) on 8 Trainium2 NeuronCores.

Strategy (data-parallel over rows, per the spec sharding hint):
  - Shard x/y row-wise across 8 cores (32768 rows each). The host ships a
    bf16 copy of x (the 2e-2 rel-err gate leaves bf16's ~1e-3 noise a 10x
    margin), tiny bf16 one-hot encodings of y, and a 128x128 identity.
  - The whole bf16 x (16MB/core = 128KB/partition) stays RESIDENT in SBUF
    across both passes, so x is read from HBM exactly once.
  - Pass 1 (stats, ~55us, TensorE-bound): stream the bf16 x in 512-row
    "quad" DMAs; square on DVE/ScalarE; accumulate per-domain
    s1 = onehot^T @ x and s2 = onehot^T @ x^2 on the TensorEngine into PSUM
    (pairs of 128-row tiles share one matmul via a 40-wide one-hot with the
    high half at partition offset 32 - PSUM reads must start at 0 mod 32).
    Counts accumulate on DVE with one final matmul.
  - AllReduce the [8, 513] partials (s1 | s2 | count) across the 8 cores.
  - Per-domain coefficients with the B-term folded into a pre-multiply
    shift: A = gamma*rsqrt(var+eps), Dsh = beta/A - mean, so that
    out = A[y] * (x + Dsh[y]) == (x - mean[y])*istd*gamma + beta.
    Identity domains (count<2): A=1, Dsh=0. Both tables in bf16.
  - Pass 2 (normalize, ~95us, DMA-write-bound): per 512-row quad, TensorE
    computes psum_x = I128 @ xb + onehot^T @ Dsh (the elementwise shift
    add costs zero DVE work - it rides the PSUM accumulator) and
    psum_a = onehot^T @ A; ScalarE copies psum_a -> SBUF; the VectorE does
    the single multiply out = psum_x * a_sb quad-wide; stream out (32MB).

Total HBM traffic ~50MB/core (16MB bf16 x in + 32MB f32 out + ~2MB
one-hots), vs ~78MB for a two-read design. bf16 only ever touches
(a) stats inputs, where rounding error averages out across ~32k samples
per domain, (b) exact 0/1 one-hot and identity weights, and (c) the A/Dsh
tables (|Dsh|~1e-2 so its absolute error is ~4e-5). Output matches the
fp32 reference to ~1.5e-3 relative - far inside the 2e-2 gate.
"""

import sys

if "/opt/trn_rl_repo" not in sys.path:
    sys.path.insert(0, "/opt/trn_rl_repo")

import numpy as np
import ml_dtypes

import concourse.bass as bass
import concourse.tile as tile
from concourse import bacc, mybir
from concourse import bass_utils

F32 = mybir.dt.float32
BF16 = mybir.dt.bfloat16
AF = mybir.ActivationFunctionType
ALU = mybir.AluOpType

N = 262144
F = 256
D = 8
CORES = 8
NR = N // CORES          # rows per core
EPS = 1e-5
MW = 40                  # one-hot stationary width per pair (high half at +32)
OT_CHUNK_TILES = 8       # tiles covered per resident oT chunk


def build_program(nr=NR, num_devices=CORES, use_collective=True):
    """Build (and compile) the SPMD bass program for `nr` rows per core."""
    quads = nr // 512
    assert nr % 512 == 0
    pairs_per_quad = 2
    ot_chunks = max(1, (nr // 128) // OT_CHUNK_TILES)

    nc = bacc.Bacc(
        "TRN2",
        target_bir_lowering=False,
        debug=False,
        enable_asserts=False,
        num_devices=num_devices,
    )

    xb_d = nc.dram_tensor("xb", [nr, F], BF16, kind="ExternalInput")
    oh16_d = nc.dram_tensor(
        "oh16", [128, (nr // 256) * MW], BF16, kind="ExternalInput"
    )
    oT_d = nc.dram_tensor("oT", [D, nr], BF16, kind="ExternalInput")
    ident_d = nc.dram_tensor("ident", [128, 128], BF16, kind="ExternalInput")
    gamma_d = nc.dram_tensor("gamma", [D, F], F32, kind="ExternalInput")
    beta_d = nc.dram_tensor("beta", [D, F], F32, kind="ExternalInput")
    out_d = nc.dram_tensor("out", [nr, F], F32, kind="ExternalOutput")

    def quad_ap(dram, q):
        # [512 rows, F] -> [128 partitions, 4, 256]: [:, j, :] = row q*512+j*128+p
        return dram[q * 512 : (q + 1) * 512, :].rearrange(
            "(four p) f -> p four f", four=4
        )

    def as4d(sbuf_ap):
        return sbuf_ap.rearrange("p (four f) -> p four f", four=4)

    with tile.TileContext(nc) as tc:
        with (
            tc.tile_pool(name="resident", bufs=1) as resident,
            tc.tile_pool(name="otc", bufs=2) as ot_pool,
            tc.tile_pool(name="xbres", bufs=1) as xbres_pool,
            tc.tile_pool(name="xx", bufs=3) as xx_pool,
            tc.tile_pool(name="asb", bufs=2) as asb_pool,
            tc.tile_pool(name="outp", bufs=3) as out_pool,
            tc.tile_pool(name="smalls", bufs=1) as smalls,
            tc.tile_pool(name="dram", bufs=1, space="DRAM") as dram,
        ):
            # ---- resident inputs ----
            oh_cols = (nr // 256) * MW
            oh16 = resident.tile([128, oh_cols], BF16)
            nc.sync.dma_start(oh16[:], oh16_d[:, :])

            def oh16_slice(g):
                return oh16[:, g * MW : (g + 1) * MW]

            ident = resident.tile([128, 128], BF16)
            nc.sync.dma_start(ident[:], ident_d[:, :])
            gsb = smalls.tile([D, F], F32)
            nc.sync.dma_start(gsb[:], gamma_d[:, :])
            bsb = smalls.tile([D, F], F32)
            nc.sync.dma_start(bsb[:], beta_d[:, :])
            xbres = [
                xbres_pool.tile([128, 1024], BF16, name=f"xbres{i}")
                for i in range(quads)
            ]
            ones_col = smalls.tile([128, 1], F32)
            nc.vector.memset(ones_col[:], 1.0)
            oh_acc = smalls.tile([128, MW], F32)
            nc.vector.memset(oh_acc[:], 0.0)

            # ---- pass 1: per-domain partial sums ----
            stats = smalls.tile([D, 513], F32)
            with tc.tile_pool(
                name="psum_stats", bufs=1, space="PSUM"
            ) as psum_stats:
                psum_A = psum_stats.tile([MW, 512], F32)
                psum_B = psum_stats.tile([MW, 512], F32)
                psum_c = psum_stats.tile([MW, 1], F32)

                for q in range(quads):
                    xbq = xbres[q]
                    nc.sync.dma_start(as4d(xbq[:]), quad_ap(xb_d, q))
                    xx = xx_pool.tile([128, 1024], BF16)
                    if q % 2 == 0:
                        nc.scalar.activation(xx[:], xbq[:], AF.Square)
                    else:
                        nc.vector.tensor_mul(xx[:], xbq[:], xbq[:])
                    # accumulate one-hot columns for counts on DVE (PE slack)
                    nc.vector.tensor_add(
                        oh_acc[:], oh_acc[:], oh16_slice(q * 2)
                    )
                    nc.vector.tensor_add(
                        oh_acc[:], oh_acc[:], oh16_slice(q * 2 + 1)
                    )
                    for hp in range(pairs_per_quad):
                        g = q * 2 + hp
                        lhsT = oh16_slice(g)
                        first = g == 0
                        last = g == 2 * quads - 1
                        nc.tensor.matmul(
                            psum_A[:],
                            lhsT,
                            xbq[:, hp * 512 : (hp + 1) * 512],
                            start=first,
                            stop=last,
                        )
                        nc.tensor.matmul(
                            psum_B[:],
                            lhsT,
                            xx[:, hp * 512 : (hp + 1) * 512],
                            start=first,
                            stop=last,
                        )

                nc.tensor.matmul(
                    psum_c[:], oh_acc[:], ones_col[:], start=True, stop=True
                )

                # fold low/high quadrants -> [8, 513] partial stats
                # (PSUM has a single DVE read port: copy one quadrant out first)
                nc.vector.tensor_copy(stats[:, 0:256], psum_A[0:8, 0:256])
                nc.vector.tensor_add(
                    stats[:, 0:256], stats[:, 0:256], psum_A[32:40, 256:512]
                )
                nc.vector.tensor_copy(stats[:, 256:512], psum_B[0:8, 0:256])
                nc.vector.tensor_add(
                    stats[:, 256:512], stats[:, 256:512], psum_B[32:40, 256:512]
                )
                nc.vector.tensor_copy(stats[:, 512:513], psum_c[0:8, :])
                nc.vector.tensor_add(
                    stats[:, 512:513], stats[:, 512:513], psum_c[32:40, :]
                )

            # ---- all-reduce partials across cores ----
            gstats = smalls.tile([D, 513], F32)
            if use_collective:
                cc_in = dram.tile([D, 513], F32)
                cc_space = "Shared" if num_devices > 4 else "Local"
                cc_out = dram.tile([D, 513], F32, addr_space=cc_space)
                nc.sync.dma_start(cc_in[:], stats[:])
                nc.gpsimd.collective_compute(
                    "AllReduce",
                    ALU.add,
                    replica_groups=[list(range(num_devices))],
                    ins=[cc_in.opt()],
                    outs=[cc_out.opt()],
                )
                nc.sync.dma_start(gstats[:], cc_out[:])
            else:
                nc.vector.tensor_copy(gstats[:], stats[:])

            # ---- per-domain affine coefficients ----
            cnt = smalls.tile([D, 1], F32)
            nc.vector.tensor_scalar_max(cnt[:], gstats[:, 512:513], 1.0)
            rc = smalls.tile([D, 1], F32)
            nc.vector.reciprocal(rc[:], cnt[:])
            mean = smalls.tile([D, F], F32)
            nc.vector.tensor_scalar_mul(mean[:], gstats[:, 0:256], rc[:])
            var = smalls.tile([D, F], F32)
            nc.vector.tensor_scalar_mul(var[:], gstats[:, 256:512], rc[:])
            m2 = smalls.tile([D, F], F32)
            nc.vector.tensor_mul(m2[:], mean[:], mean[:])
            nc.vector.tensor_sub(var[:], var[:], m2[:])
            # fp roundoff can leave var a hair negative when true var == 0
            nc.vector.tensor_scalar_max(var[:], var[:], 0.0)
            eps_ap = smalls.tile([D, 1], F32)
            nc.vector.memset(eps_ap[:], EPS)
            std = smalls.tile([D, F], F32)
            nc.scalar.activation(std[:], var[:], AF.Sqrt, bias=eps_ap[:])
            istd = smalls.tile([D, F], F32)
            nc.vector.reciprocal(istd[:], std[:])
            # use_bn mask: 1.0 where count > 1 else 0.0
            mask = smalls.tile([D, 1], F32)
            nc.vector.tensor_scalar(
                mask[:], gstats[:, 512:513], 1.0, None, op0=ALU.is_gt
            )

            # A = ((gamma*istd) - 1)*mask + 1 ; Dsh = (beta/A - mean)*mask
            a_f = smalls.tile([D, 256], F32)
            nc.vector.tensor_mul(a_f[:], gsb[:], istd[:])
            nc.vector.tensor_scalar_add(a_f[:], a_f[:], -1.0)
            nc.vector.tensor_scalar_mul(a_f[:], a_f[:], mask[:])
            nc.vector.tensor_scalar_add(a_f[:], a_f[:], 1.0)
            ra_f = smalls.tile([D, 256], F32)
            nc.vector.tensor_scalar_max(ra_f[:], a_f[:], 1e-20)
            nc.vector.reciprocal(ra_f[:], ra_f[:])
            d_f = smalls.tile([D, 256], F32)
            nc.vector.tensor_mul(d_f[:], bsb[:], ra_f[:])
            nc.vector.tensor_sub(d_f[:], d_f[:], mean[:])
            nc.vector.tensor_scalar_mul(d_f[:], d_f[:], mask[:])

            a16 = smalls.tile([D, 256], BF16)
            nc.vector.tensor_copy(a16[:], a_f[:])
            d16 = smalls.tile([D, 256], BF16)
            nc.vector.tensor_copy(d16[:], d_f[:])

            # ---- pass 2: normalize ----
            psum_x_pool = tc.alloc_tile_pool(name="psum_x", bufs=2, space="PSUM")
            psum_a_pool = tc.alloc_tile_pool(name="psum_a", bufs=2, space="PSUM")
            ot_tiles_per_chunk = (nr // 128) // ot_chunks

            def get_ot_chunk(c, cache={}):
                if c not in cache:
                    otc = ot_pool.tile([D, ot_tiles_per_chunk * 128], BF16)
                    nc.sync.dma_start(
                        otc[:],
                        oT_d[
                            :,
                            c * ot_tiles_per_chunk * 128 : (c + 1)
                            * ot_tiles_per_chunk
                            * 128,
                        ],
                    )
                    cache[c] = otc
                return cache[c]

            for q in range(quads):
                psum_x = psum_x_pool.tile([128, 1024], F32)
                psum_a = psum_a_pool.tile([128, 1024], F32)
                # x rides the PSUM accumulator via an identity matmul; the
                # per-row Dsh shift accumulates on top from a one-hot gather
                nc.tensor.matmul(
                    psum_x[:],
                    ident[:],
                    xbres[q][:],
                    start=True,
                    stop=False,
                    skip_group_check=True,
                )
                for j in range(4):
                    t = q * 4 + j
                    otc = get_ot_chunk(t // ot_tiles_per_chunk)
                    r = t % ot_tiles_per_chunk
                    lhsT = otc[:, r * 128 : (r + 1) * 128]
                    nc.tensor.matmul(
                        psum_x[:, j * 256 : (j + 1) * 256],
                        lhsT,
                        d16[:],
                        start=False,
                        stop=True,
                        skip_group_check=True,
                    )
                    nc.tensor.matmul(
                        psum_a[:, j * 256 : (j + 1) * 256],
                        lhsT,
                        a16[:],
                        start=True,
                        stop=True,
                        skip_group_check=True,
                    )
                a_sb = asb_pool.tile([128, 1024], F32)
                nc.scalar.activation(a_sb[:], psum_a[:], AF.Copy)
                outp = out_pool.tile([128, 1024], F32)
                nc.vector.tensor_mul(outp[:], psum_x[:], a_sb[:])
                nc.scalar.dma_start(quad_ap(out_d, q), as4d(outp[:]))
            psum_a_pool.release()
            psum_x_pool.release()

    nc.compile()
    return nc


def host_prep(x, y, gamma, beta, nr=NR, num_devices=CORES):
    """Shard + encode inputs per core."""
    x = np.ascontiguousarray(np.asarray(x, dtype=np.float32))
    y = np.asarray(y, dtype=np.int32)
    gamma = np.ascontiguousarray(np.asarray(gamma, dtype=np.float32))
    beta = np.ascontiguousarray(np.asarray(beta, dtype=np.float32))
    dom = np.arange(D, dtype=np.int32)
    ident = np.eye(128, dtype=ml_dtypes.bfloat16)
    in_maps = []
    for c in range(num_devices):
        ys = y[c * nr : (c + 1) * nr]
        pairs = nr // 256
        ohw = np.zeros((pairs, 128, MW), dtype=ml_dtypes.bfloat16)
        yp = ys.reshape(pairs, 2, 128)
        ohw[:, :, 0:8] = yp[:, 0, :, None] == dom
        ohw[:, :, 32:40] = yp[:, 1, :, None] == dom
        oh16 = np.ascontiguousarray(ohw.transpose(1, 0, 2).reshape(128, -1))
        oT = np.ascontiguousarray((ys[None, :] == dom[:, None])).astype(
            ml_dtypes.bfloat16
        )
        xs = x[c * nr : (c + 1) * nr]
        xbs = xs.astype(ml_dtypes.bfloat16)
        in_maps.append(
            {
                "xb": xbs,
                "oh16": oh16,
                "oT": oT,
                "ident": ident,
                "gamma": gamma,
                "beta": beta,
            }
        )
    return in_maps


_CACHE = {}


def _get_program():
    if "nc" not in _CACHE:
        _CACHE["nc"] = build_program()
    return _CACHE["nc"]


def kernel(x, y, gamma, beta):
    nc = _get_program()
    in_maps = host_prep(x, y, gamma, beta)
    res = bass_utils.run_bass_kernel_spmd(nc, in_maps, core_ids=list(range(CORES)))
    out = np.empty((N, F), dtype=np.float32)
    for c in range(CORES):
        out[c * NR : (c + 1) * NR] = res.results[c]["out"]
    return out


# revision 7
# speedup vs baseline: 1.1872x; 1.0279x over previous
"""Domain-specific BatchNorm (8 domains) on 8 Trainium2 NeuronCores.

Strategy (data-parallel over rows, per the spec sharding hint):
  - Shard x/y row-wise across 8 cores (32768 rows each). The host ships a
    bf16 copy of x (the 2e-2 rel-err gate leaves bf16's ~1e-3 noise a 10x
    margin), tiny bf16 one-hot encodings of y, and a 128x128 identity.
  - The whole bf16 x (16MB/core = 128KB/partition) stays RESIDENT in SBUF
    across both passes, so x is read from HBM exactly once.
  - Pass 1 (stats, ~55us, TensorE-bound): stream the bf16 x in 512-row
    "quad" DMAs; square on DVE/ScalarE; accumulate per-domain
    s1 = onehot^T @ x and s2 = onehot^T @ x^2 on the TensorEngine into PSUM
    (pairs of 128-row tiles share one matmul via a 40-wide one-hot with the
    high half at partition offset 32 - PSUM reads must start at 0 mod 32).
    Counts accumulate on DVE with one final matmul.
  - AllReduce the [8, 513] partials (s1 | s2 | count) across the 8 cores.
  - Per-domain coefficients with the B-term folded into a pre-multiply
    shift: A = gamma*rsqrt(var+eps), Dsh = beta/A - mean, so that
    out = A[y] * (x + Dsh[y]) == (x - mean[y])*istd*gamma + beta.
    Identity domains (count<2): A=1, Dsh=0. Both tables in bf16.
  - Pass 2 (normalize, ~95us, DMA-write-bound): per 512-row quad, TensorE
    computes psum_x = I128 @ xb + onehot^T @ Dsh (the elementwise shift
    add costs zero DVE work - it rides the PSUM accumulator) and
    psum_a = onehot^T @ A; ScalarE copies psum_a -> SBUF; the VectorE does
    the single multiply out = psum_x * a_sb quad-wide; stream out (32MB).

Total HBM traffic ~50MB/core (16MB bf16 x in + 32MB f32 out + ~2MB
one-hots), vs ~78MB for a two-read design. bf16 only ever touches
(a) stats inputs, where rounding error averages out across ~32k samples
per domain, (b) exact 0/1 one-hot and identity weights, and (c) the A/Dsh
tables (|Dsh|~1e-2 so its absolute error is ~4e-5). Output matches the
fp32 reference to ~1.5e-3 relative - far inside the 2e-2 gate.
"""

import sys

if "/opt/trn_rl_repo" not in sys.path:
    sys.path.insert(0, "/opt/trn_rl_repo")

import numpy as np
import ml_dtypes

import concourse.bass as bass
import concourse.tile as tile
from concourse import bacc, mybir
from concourse import bass_utils

F32 = mybir.dt.float32
BF16 = mybir.dt.bfloat16
AF = mybir.ActivationFunctionType
ALU = mybir.AluOpType

N = 262144
F = 256
D = 8
CORES = 8
NR = N // CORES          # rows per core
EPS = 1e-5
MW = 40                  # one-hot stationary width per pair (high half at +32)
OT_CHUNK_TILES = 8       # tiles covered per resident oT chunk


def build_program(nr=NR, num_devices=CORES, use_collective=True):
    """Build (and compile) the SPMD bass program for `nr` rows per core."""
    quads = nr // 512
    assert nr % 512 == 0
    pairs_per_quad = 2
    ot_chunks = max(1, (nr // 128) // OT_CHUNK_TILES)

    nc = bacc.Bacc(
        "TRN2",
        target_bir_lowering=False,
        debug=False,
        enable_asserts=False,
        num_devices=num_devices,
    )

    xb_d = nc.dram_tensor("xb", [nr, F], BF16, kind="ExternalInput")
    oh16_d = nc.dram_tensor(
        "oh16", [128, (nr // 256) * MW], BF16, kind="ExternalInput"
    )
    oT_d = nc.dram_tensor("oT", [D, nr], BF16, kind="ExternalInput")
    ident_d = nc.dram_tensor("ident", [128, 128], BF16, kind="ExternalInput")
    gamma_d = nc.dram_tensor("gamma", [D, F], F32, kind="ExternalInput")
    beta_d = nc.dram_tensor("beta", [D, F], F32, kind="ExternalInput")
    out_d = nc.dram_tensor("out", [nr, F], F32, kind="ExternalOutput")

    def quad_ap(dram, q):
        # [512 rows, F] -> [128 partitions, 4, 256]: [:, j, :] = row q*512+j*128+p
        return dram[q * 512 : (q + 1) * 512, :].rearrange(
            "(four p) f -> p four f", four=4
        )

    def as4d(sbuf_ap):
        return sbuf_ap.rearrange("p (four f) -> p four f", four=4)

    with tile.TileContext(nc) as tc:
        with (
            tc.tile_pool(name="resident", bufs=1) as resident,
            tc.tile_pool(name="otc", bufs=4) as ot_pool,
            tc.tile_pool(name="xbres", bufs=1) as xbres_pool,
            tc.tile_pool(name="xx", bufs=3) as xx_pool,
            tc.tile_pool(name="asb", bufs=6) as asb_pool,
            tc.tile_pool(name="outp", bufs=4) as out_pool,
            tc.tile_pool(name="smalls", bufs=1) as smalls,
            tc.tile_pool(name="dram", bufs=1, space="DRAM") as dram,
        ):
            # ---- resident inputs ----
            oh_cols = (nr // 256) * MW
            oh16 = resident.tile([128, oh_cols], BF16)
            nc.sync.dma_start(oh16[:], oh16_d[:, :])

            def oh16_slice(g):
                return oh16[:, g * MW : (g + 1) * MW]

            ident = resident.tile([128, 128], BF16)
            nc.sync.dma_start(ident[:], ident_d[:, :])
            gsb = smalls.tile([D, F], F32)
            nc.sync.dma_start(gsb[:], gamma_d[:, :])
            bsb = smalls.tile([D, F], F32)
            nc.sync.dma_start(bsb[:], beta_d[:, :])
            xbres = [
                xbres_pool.tile([128, 1024], BF16, name=f"xbres{i}")
                for i in range(quads)
            ]
            ones_col = smalls.tile([128, 1], F32)
            nc.vector.memset(ones_col[:], 1.0)
            oh_acc = smalls.tile([128, MW], F32)
            nc.vector.memset(oh_acc[:], 0.0)

            # ---- pass 1: per-domain partial sums ----
            stats = smalls.tile([D, 513], F32)
            with tc.tile_pool(
                name="psum_stats", bufs=1, space="PSUM"
            ) as psum_stats:
                psum_A = psum_stats.tile([MW, 512], F32)
                psum_B = psum_stats.tile([MW, 512], F32)
                psum_c = psum_stats.tile([MW, 1], F32)

                for q in range(quads):
                    xbq = xbres[q]
                    nc.sync.dma_start(as4d(xbq[:]), quad_ap(xb_d, q))
                    xx = xx_pool.tile([128, 1024], BF16)
                    if q % 2 == 0:
                        nc.scalar.activation(xx[:], xbq[:], AF.Square)
                    else:
                        nc.vector.tensor_mul(xx[:], xbq[:], xbq[:])
                    # accumulate one-hot columns for counts on DVE (PE slack)
                    nc.vector.tensor_add(
                        oh_acc[:], oh_acc[:], oh16_slice(q * 2)
                    )
                    nc.vector.tensor_add(
                        oh_acc[:], oh_acc[:], oh16_slice(q * 2 + 1)
                    )
                    for hp in range(pairs_per_quad):
                        g = q * 2 + hp
                        lhsT = oh16_slice(g)
                        first = g == 0
                        last = g == 2 * quads - 1
                        nc.tensor.matmul(
                            psum_A[:],
                            lhsT,
                            xbq[:, hp * 512 : (hp + 1) * 512],
                            start=first,
                            stop=last,
                        )
                        nc.tensor.matmul(
                            psum_B[:],
                            lhsT,
                            xx[:, hp * 512 : (hp + 1) * 512],
                            start=first,
                            stop=last,
                        )

                nc.tensor.matmul(
                    psum_c[:], oh_acc[:], ones_col[:], start=True, stop=True
                )

                # fold low/high quadrants -> [8, 513] partial stats
                # (PSUM has a single DVE read port: copy one quadrant out first)
                nc.vector.tensor_copy(stats[:, 0:256], psum_A[0:8, 0:256])
                nc.vector.tensor_add(
                    stats[:, 0:256], stats[:, 0:256], psum_A[32:40, 256:512]
                )
                nc.vector.tensor_copy(stats[:, 256:512], psum_B[0:8, 0:256])
                nc.vector.tensor_add(
                    stats[:, 256:512], stats[:, 256:512], psum_B[32:40, 256:512]
                )
                nc.vector.tensor_copy(stats[:, 512:513], psum_c[0:8, :])
                nc.vector.tensor_add(
                    stats[:, 512:513], stats[:, 512:513], psum_c[32:40, :]
                )

            # ---- all-reduce partials across cores ----
            gstats = smalls.tile([D, 513], F32)
            if use_collective:
                cc_in = dram.tile([D, 513], F32)
                cc_space = "Shared" if num_devices > 4 else "Local"
                cc_out = dram.tile([D, 513], F32, addr_space=cc_space)
                nc.sync.dma_start(cc_in[:], stats[:])
                nc.gpsimd.collective_compute(
                    "AllReduce",
                    ALU.add,
                    replica_groups=[list(range(num_devices))],
                    ins=[cc_in.opt()],
                    outs=[cc_out.opt()],
                )
                nc.sync.dma_start(gstats[:], cc_out[:])
            else:
                nc.vector.tensor_copy(gstats[:], stats[:])

            # ---- per-domain affine coefficients ----
            cnt = smalls.tile([D, 1], F32)
            nc.vector.tensor_scalar_max(cnt[:], gstats[:, 512:513], 1.0)
            rc = smalls.tile([D, 1], F32)
            nc.vector.reciprocal(rc[:], cnt[:])
            mean = smalls.tile([D, F], F32)
            nc.vector.tensor_scalar_mul(mean[:], gstats[:, 0:256], rc[:])
            var = smalls.tile([D, F], F32)
            nc.vector.tensor_scalar_mul(var[:], gstats[:, 256:512], rc[:])
            m2 = smalls.tile([D, F], F32)
            nc.vector.tensor_mul(m2[:], mean[:], mean[:])
            nc.vector.tensor_sub(var[:], var[:], m2[:])
            # fp roundoff can leave var a hair negative when true var == 0
            nc.vector.tensor_scalar_max(var[:], var[:], 0.0)
            eps_ap = smalls.tile([D, 1], F32)
            nc.vector.memset(eps_ap[:], EPS)
            std = smalls.tile([D, F], F32)
            nc.scalar.activation(std[:], var[:], AF.Sqrt, bias=eps_ap[:])
            istd = smalls.tile([D, F], F32)
            nc.vector.reciprocal(istd[:], std[:])
            # use_bn mask: 1.0 where count > 1 else 0.0
            mask = smalls.tile([D, 1], F32)
            nc.vector.tensor_scalar(
                mask[:], gstats[:, 512:513], 1.0, None, op0=ALU.is_gt
            )

            # A = ((gamma*istd) - 1)*mask + 1 ; Dsh = (beta/A - mean)*mask
            a_f = smalls.tile([D, 256], F32)
            nc.vector.tensor_mul(a_f[:], gsb[:], istd[:])
            nc.vector.tensor_scalar_add(a_f[:], a_f[:], -1.0)
            nc.vector.tensor_scalar_mul(a_f[:], a_f[:], mask[:])
            nc.vector.tensor_scalar_add(a_f[:], a_f[:], 1.0)
            ra_f = smalls.tile([D, 256], F32)
            nc.vector.tensor_scalar_max(ra_f[:], a_f[:], 1e-20)
            nc.vector.reciprocal(ra_f[:], ra_f[:])
            d_f = smalls.tile([D, 256], F32)
            nc.vector.tensor_mul(d_f[:], bsb[:], ra_f[:])
            nc.vector.tensor_sub(d_f[:], d_f[:], mean[:])
            nc.vector.tensor_scalar_mul(d_f[:], d_f[:], mask[:])

            a16 = smalls.tile([D, 256], BF16)
            nc.vector.tensor_copy(a16[:], a_f[:])
            d16 = smalls.tile([D, 256], BF16)
            nc.vector.tensor_copy(d16[:], d_f[:])

            # ---- pass 2: normalize ----
            psum_x_pool = tc.alloc_tile_pool(name="psum_x", bufs=4, space="PSUM")
            psum_a_pool = tc.alloc_tile_pool(name="psum_a", bufs=4, space="PSUM")
            ot_tiles_per_chunk = (nr // 128) // ot_chunks

            def get_ot_chunk(c, cache={}):
                if c not in cache:
                    otc = ot_pool.tile([D, ot_tiles_per_chunk * 128], BF16)
                    nc.sync.dma_start(
                        otc[:],
                        oT_d[
                            :,
                            c * ot_tiles_per_chunk * 128 : (c + 1)
                            * ot_tiles_per_chunk
                            * 128,
                        ],
                    )
                    cache[c] = otc
                return cache[c]

            # pair-granularity (256-row) pipeline: PSUM tiles are 1 bank each
            # so bufs=4 gives enough depth to cover the PE->Act->DVE->DMA
            # chain latency without any engine stalling
            outp = None
            for h in range(2 * quads):
                q, half = h // 2, h % 2
                psum_x = psum_x_pool.tile([128, 512], F32)
                psum_a = psum_a_pool.tile([128, 512], F32)
                # x rides the PSUM accumulator via an identity matmul; the
                # per-row Dsh shift accumulates on top from a one-hot gather
                nc.tensor.matmul(
                    psum_x[:],
                    ident[:],
                    xbres[q][:, half * 512 : (half + 1) * 512],
                    start=True,
                    stop=False,
                    skip_group_check=True,
                )
                for j in range(2):
                    t = h * 2 + j
                    otc = get_ot_chunk(t // ot_tiles_per_chunk)
                    r = t % ot_tiles_per_chunk
                    lhsT = otc[:, r * 128 : (r + 1) * 128]
                    nc.tensor.matmul(
                        psum_x[:, j * 256 : (j + 1) * 256],
                        lhsT,
                        d16[:],
                        start=False,
                        stop=True,
                        skip_group_check=True,
                    )
                    nc.tensor.matmul(
                        psum_a[:, j * 256 : (j + 1) * 256],
                        lhsT,
                        a16[:],
                        start=True,
                        stop=True,
                        skip_group_check=True,
                    )
                a_sb = asb_pool.tile([128, 512], F32)
                nc.scalar.activation(a_sb[:], psum_a[:], AF.Copy)
                if half == 0:
                    outp = out_pool.tile([128, 1024], F32, name="outp")
                nc.vector.tensor_mul(
                    outp[:, half * 512 : (half + 1) * 512], psum_x[:], a_sb[:]
                )
                if half == 1:
                    nc.scalar.dma_start(quad_ap(out_d, q), as4d(outp[:]))
            psum_a_pool.release()
            psum_x_pool.release()

    nc.compile()
    return nc


def host_prep(x, y, gamma, beta, nr=NR, num_devices=CORES):
    """Shard + encode inputs per core."""
    x = np.ascontiguousarray(np.asarray(x, dtype=np.float32))
    y = np.asarray(y, dtype=np.int32)
    gamma = np.ascontiguousarray(np.asarray(gamma, dtype=np.float32))
    beta = np.ascontiguousarray(np.asarray(beta, dtype=np.float32))
    dom = np.arange(D, dtype=np.int32)
    ident = np.eye(128, dtype=ml_dtypes.bfloat16)
    in_maps = []
    for c in range(num_devices):
        ys = y[c * nr : (c + 1) * nr]
        pairs = nr // 256
        ohw = np.zeros((pairs, 128, MW), dtype=ml_dtypes.bfloat16)
        yp = ys.reshape(pairs, 2, 128)
        ohw[:, :, 0:8] = yp[:, 0, :, None] == dom
        ohw[:, :, 32:40] = yp[:, 1, :, None] == dom
        oh16 = np.ascontiguousarray(ohw.transpose(1, 0, 2).reshape(128, -1))
        oT = np.ascontiguousarray((ys[None, :] == dom[:, None])).astype(
            ml_dtypes.bfloat16
        )
        xs = x[c * nr : (c + 1) * nr]
        xbs = xs.astype(ml_dtypes.bfloat16)
        in_maps.append(
            {
                "xb": xbs,
                "oh16": oh16,
                "oT": oT,
                "ident": ident,
                "gamma": gamma,
                "beta": beta,
            }
        )
    return in_maps


_CACHE = {}


def _get_program():
    if "nc" not in _CACHE:
        _CACHE["nc"] = build_program()
    return _CACHE["nc"]


def kernel(x, y, gamma, beta):
    nc = _get_program()
    in_maps = host_prep(x, y, gamma, beta)
    res = bass_utils.run_bass_kernel_spmd(nc, in_maps, core_ids=list(range(CORES)))
    out = np.empty((N, F), dtype=np.float32)
    for c in range(CORES):
        out[c * NR : (c + 1) * NR] = res.results[c]["out"]
    return out


# revision 8
# speedup vs baseline: 1.4787x; 1.2456x over previous
"""Domain-specific BatchNorm (8 domains) on 8 Trainium2 NeuronCores.

Strategy (data-parallel over rows, per the spec sharding hint):
  - Shard x/y row-wise across 8 cores (32768 rows each). The host ships a
    bf16 copy of x (the 2e-2 rel-err gate leaves bf16's ~1e-3 noise a 10x
    margin), tiny bf16 one-hot encodings of y, and a 128x128 identity.
  - The whole bf16 x (16MB/core = 128KB/partition) stays RESIDENT in SBUF
    across both passes, so x is read from HBM exactly once.
  - Pass 1 (stats, ~55us, TensorE-bound): stream the bf16 x in 512-row
    "quad" DMAs; square on DVE/ScalarE; accumulate per-domain
    s1 = onehot^T @ x and s2 = onehot^T @ x^2 on the TensorEngine into PSUM
    (pairs of 128-row tiles share one matmul via a 40-wide one-hot with the
    high half at partition offset 32 - PSUM reads must start at 0 mod 32).
    Counts accumulate on DVE with one final matmul.
  - AllReduce the [8, 513] partials (s1 | s2 | count) across the 8 cores.
  - Per-domain coefficients with the B-term folded into a pre-multiply
    shift: A = gamma*rsqrt(var+eps), Dsh = beta/A - mean, so that
    out = A[y] * (x + Dsh[y]) == (x - mean[y])*istd*gamma + beta.
    Identity domains (count<2): A=1, Dsh=0. Both tables in bf16.
  - Pass 2 (normalize, ~95us, DMA-write-bound): per 512-row quad, TensorE
    computes psum_x = I128 @ xb + onehot^T @ Dsh (the elementwise shift
    add costs zero DVE work - it rides the PSUM accumulator) and
    psum_a = onehot^T @ A; ScalarE copies psum_a -> SBUF; the VectorE does
    the single multiply out = psum_x * a_sb quad-wide; stream out (32MB).

Total HBM traffic ~50MB/core (16MB bf16 x in + 32MB f32 out + ~2MB
one-hots), vs ~78MB for a two-read design. bf16 only ever touches
(a) stats inputs, where rounding error averages out across ~32k samples
per domain, (b) exact 0/1 one-hot and identity weights, and (c) the A/Dsh
tables (|Dsh|~1e-2 so its absolute error is ~4e-5). Output matches the
fp32 reference to ~1.5e-3 relative - far inside the 2e-2 gate.
"""

import sys

if "/opt/trn_rl_repo" not in sys.path:
    sys.path.insert(0, "/opt/trn_rl_repo")

import numpy as np
import ml_dtypes

import concourse.bass as bass
import concourse.tile as tile
from concourse import bacc, mybir
from concourse import bass_utils

F32 = mybir.dt.float32
BF16 = mybir.dt.bfloat16
AF = mybir.ActivationFunctionType
ALU = mybir.AluOpType

N = 262144
F = 256
D = 8
CORES = 8
NR = N // CORES          # rows per core
EPS = 1e-5
MW = 40                  # one-hot stationary width per pair (high half at +32)
OT_CHUNK_TILES = 8       # tiles covered per resident oT chunk


def build_program(nr=NR, num_devices=CORES, use_collective=True):
    """Build (and compile) the SPMD bass program for `nr` rows per core."""
    quads = nr // 512
    assert nr % 512 == 0
    pairs_per_quad = 2
    ot_chunks = max(1, (nr // 128) // OT_CHUNK_TILES)

    nc = bacc.Bacc(
        "TRN2",
        target_bir_lowering=False,
        debug=False,
        enable_asserts=False,
        num_devices=num_devices,
    )

    xb_d = nc.dram_tensor("xb", [nr, F], BF16, kind="ExternalInput")
    oh16_d = nc.dram_tensor(
        "oh16", [128, (nr // 256) * MW], BF16, kind="ExternalInput"
    )
    oT_d = nc.dram_tensor("oT", [D, nr], BF16, kind="ExternalInput")
    ident_d = nc.dram_tensor("ident", [128, 128], BF16, kind="ExternalInput")
    gamma_d = nc.dram_tensor("gamma", [D, F], F32, kind="ExternalInput")
    beta_d = nc.dram_tensor("beta", [D, F], F32, kind="ExternalInput")
    out_d = nc.dram_tensor("out", [nr, F], F32, kind="ExternalOutput")

    def quad_ap(dram, q):
        # [512 rows, F] -> [128 partitions, 4, 256]: [:, j, :] = row q*512+j*128+p
        return dram[q * 512 : (q + 1) * 512, :].rearrange(
            "(four p) f -> p four f", four=4
        )

    def as4d(sbuf_ap):
        return sbuf_ap.rearrange("p (four f) -> p four f", four=4)

    with tile.TileContext(nc) as tc:
        with (
            tc.tile_pool(name="resident", bufs=1) as resident,
            tc.tile_pool(name="otc", bufs=4) as ot_pool,
            tc.tile_pool(name="xbres", bufs=1) as xbres_pool,
            tc.tile_pool(name="xx", bufs=3) as xx_pool,
            tc.tile_pool(name="asb", bufs=6) as asb_pool,
            tc.tile_pool(name="outp", bufs=4) as out_pool,
            tc.tile_pool(name="smalls", bufs=1) as smalls,
            tc.tile_pool(name="dram", bufs=1, space="DRAM") as dram,
        ):
            # ---- resident inputs ----
            oh_cols = (nr // 256) * MW
            oh16 = resident.tile([128, oh_cols], BF16)
            nc.sync.dma_start(oh16[:], oh16_d[:, :])

            def oh16_slice(g):
                return oh16[:, g * MW : (g + 1) * MW]

            ident = resident.tile([128, 128], BF16)
            nc.sync.dma_start(ident[:], ident_d[:, :])
            gsb = smalls.tile([D, F], F32)
            nc.sync.dma_start(gsb[:], gamma_d[:, :])
            bsb = smalls.tile([D, F], F32)
            nc.sync.dma_start(bsb[:], beta_d[:, :])
            xbres = [
                xbres_pool.tile([128, 1024], BF16, name=f"xbres{i}")
                for i in range(quads)
            ]
            ones_col = smalls.tile([128, 1], F32)
            nc.vector.memset(ones_col[:], 1.0)
            oh_acc = smalls.tile([128, MW], F32)
            nc.vector.memset(oh_acc[:], 0.0)

            # ---- pass 1: per-domain partial sums ----
            stats = smalls.tile([D, 513], F32)
            with tc.tile_pool(
                name="psum_stats", bufs=1, space="PSUM"
            ) as psum_stats:
                psum_A = psum_stats.tile([MW, 512], F32)
                psum_B = psum_stats.tile([MW, 512], F32)
                psum_c = psum_stats.tile([MW, 1], F32)

                for q in range(quads):
                    xbq = xbres[q]
                    nc.sync.dma_start(as4d(xbq[:]), quad_ap(xb_d, q))
                    xx = xx_pool.tile([128, 1024], BF16)
                    if q % 2 == 0:
                        nc.scalar.activation(xx[:], xbq[:], AF.Square)
                    else:
                        nc.vector.tensor_mul(xx[:], xbq[:], xbq[:])
                    # accumulate one-hot columns for counts on DVE (PE slack)
                    nc.vector.tensor_add(
                        oh_acc[:], oh_acc[:], oh16_slice(q * 2)
                    )
                    nc.vector.tensor_add(
                        oh_acc[:], oh_acc[:], oh16_slice(q * 2 + 1)
                    )
                    for hp in range(pairs_per_quad):
                        g = q * 2 + hp
                        lhsT = oh16_slice(g)
                        first = g == 0
                        last = g == 2 * quads - 1
                        nc.tensor.matmul(
                            psum_A[:],
                            lhsT,
                            xbq[:, hp * 512 : (hp + 1) * 512],
                            start=first,
                            stop=last,
                        )
                        nc.tensor.matmul(
                            psum_B[:],
                            lhsT,
                            xx[:, hp * 512 : (hp + 1) * 512],
                            start=first,
                            stop=last,
                        )

                nc.tensor.matmul(
                    psum_c[:], oh_acc[:], ones_col[:], start=True, stop=True
                )

                # fold low/high quadrants -> [8, 513] partial stats
                # (PSUM has a single DVE read port: copy one quadrant out first)
                nc.vector.tensor_copy(stats[:, 0:256], psum_A[0:8, 0:256])
                nc.vector.tensor_add(
                    stats[:, 0:256], stats[:, 0:256], psum_A[32:40, 256:512]
                )
                nc.vector.tensor_copy(stats[:, 256:512], psum_B[0:8, 0:256])
                nc.vector.tensor_add(
                    stats[:, 256:512], stats[:, 256:512], psum_B[32:40, 256:512]
                )
                nc.vector.tensor_copy(stats[:, 512:513], psum_c[0:8, :])
                nc.vector.tensor_add(
                    stats[:, 512:513], stats[:, 512:513], psum_c[32:40, :]
                )

            # ---- all-reduce partials across cores ----
            gstats = smalls.tile([D, 513], F32)
            if use_collective:
                cc_in = dram.tile([D, 513], F32)
                cc_space = "Shared" if num_devices > 4 else "Local"
                cc_out = dram.tile([D, 513], F32, addr_space=cc_space)
                nc.sync.dma_start(cc_in[:], stats[:])
                nc.gpsimd.collective_compute(
                    "AllReduce",
                    ALU.add,
                    replica_groups=[list(range(num_devices))],
                    ins=[cc_in.opt()],
                    outs=[cc_out.opt()],
                )
                nc.sync.dma_start(gstats[:], cc_out[:])
            else:
                nc.vector.tensor_copy(gstats[:], stats[:])

            # ---- per-domain affine coefficients ----
            cnt = smalls.tile([D, 1], F32)
            nc.vector.tensor_scalar_max(cnt[:], gstats[:, 512:513], 1.0)
            rc = smalls.tile([D, 1], F32)
            nc.vector.reciprocal(rc[:], cnt[:])
            mean = smalls.tile([D, F], F32)
            nc.vector.tensor_scalar_mul(mean[:], gstats[:, 0:256], rc[:])
            var = smalls.tile([D, F], F32)
            nc.vector.tensor_scalar_mul(var[:], gstats[:, 256:512], rc[:])
            m2 = smalls.tile([D, F], F32)
            nc.vector.tensor_mul(m2[:], mean[:], mean[:])
            nc.vector.tensor_sub(var[:], var[:], m2[:])
            # fp roundoff can leave var a hair negative when true var == 0
            nc.vector.tensor_scalar_max(var[:], var[:], 0.0)
            eps_ap = smalls.tile([D, 1], F32)
            nc.vector.memset(eps_ap[:], EPS)
            std = smalls.tile([D, F], F32)
            nc.scalar.activation(std[:], var[:], AF.Sqrt, bias=eps_ap[:])
            istd = smalls.tile([D, F], F32)
            nc.vector.reciprocal(istd[:], std[:])
            # use_bn mask: 1.0 where count > 1 else 0.0
            mask = smalls.tile([D, 1], F32)
            nc.vector.tensor_scalar(
                mask[:], gstats[:, 512:513], 1.0, None, op0=ALU.is_gt
            )

            # A = ((gamma*istd) - 1)*mask + 1 ; Dsh = (beta/A - mean)*mask
            a_f = smalls.tile([D, 256], F32)
            nc.vector.tensor_mul(a_f[:], gsb[:], istd[:])
            nc.vector.tensor_scalar_add(a_f[:], a_f[:], -1.0)
            nc.vector.tensor_scalar_mul(a_f[:], a_f[:], mask[:])
            nc.vector.tensor_scalar_add(a_f[:], a_f[:], 1.0)
            ra_f = smalls.tile([D, 256], F32)
            nc.vector.tensor_scalar_max(ra_f[:], a_f[:], 1e-20)
            nc.vector.reciprocal(ra_f[:], ra_f[:])
            d_f = smalls.tile([D, 256], F32)
            nc.vector.tensor_mul(d_f[:], bsb[:], ra_f[:])
            nc.vector.tensor_sub(d_f[:], d_f[:], mean[:])
            nc.vector.tensor_scalar_mul(d_f[:], d_f[:], mask[:])

            a16 = smalls.tile([D, 256], BF16)
            nc.vector.tensor_copy(a16[:], a_f[:])
            d16 = smalls.tile([D, 256], BF16)
            nc.vector.tensor_copy(d16[:], d_f[:])

            # ---- pass 2: normalize ----
            psum_x_pool = tc.alloc_tile_pool(name="psum_x", bufs=4, space="PSUM")
            psum_a_pool = tc.alloc_tile_pool(name="psum_a", bufs=4, space="PSUM")
            ot_tiles_per_chunk = (nr // 128) // ot_chunks

            def get_ot_chunk(c, cache={}):
                if c not in cache:
                    otc = ot_pool.tile([D, ot_tiles_per_chunk * 128], BF16)
                    nc.sync.dma_start(
                        otc[:],
                        oT_d[
                            :,
                            c * ot_tiles_per_chunk * 128 : (c + 1)
                            * ot_tiles_per_chunk
                            * 128,
                        ],
                    )
                    cache[c] = otc
                return cache[c]

            # pair-granularity (256-row) pipeline: PSUM tiles are 1 bank each
            # so bufs=4 gives enough depth to cover the PE->Act->DVE->DMA
            # chain latency without any engine stalling
            outp = None
            for h in range(2 * quads):
                q, half = h // 2, h % 2
                psum_x = psum_x_pool.tile([128, 512], F32)
                psum_a = psum_a_pool.tile([128, 512], F32)
                # x rides the PSUM accumulator via an identity matmul; the
                # per-row Dsh shift accumulates on top from a one-hot gather
                nc.tensor.matmul(
                    psum_x[:],
                    ident[:],
                    xbres[q][:, half * 512 : (half + 1) * 512],
                    start=True,
                    stop=False,
                    skip_group_check=True,
                )
                for j in range(2):
                    t = h * 2 + j
                    otc = get_ot_chunk(t // ot_tiles_per_chunk)
                    r = t % ot_tiles_per_chunk
                    lhsT = otc[:, r * 128 : (r + 1) * 128]
                    nc.tensor.matmul(
                        psum_x[:, j * 256 : (j + 1) * 256],
                        lhsT,
                        d16[:],
                        start=False,
                        stop=True,
                        skip_group_check=True,
                    )
                    nc.tensor.matmul(
                        psum_a[:, j * 256 : (j + 1) * 256],
                        lhsT,
                        a16[:],
                        start=True,
                        stop=True,
                        skip_group_check=True,
                    )
                a_sb = asb_pool.tile([128, 512], F32)
                nc.scalar.activation(a_sb[:], psum_a[:], AF.Copy)
                if half == 0:
                    outp = out_pool.tile([128, 1024], F32, name="outp")
                nc.vector.tensor_mul(
                    outp[:, half * 512 : (half + 1) * 512], psum_x[:], a_sb[:]
                )
                if half == 1:
                    # issue from the otherwise-idle Pool queue: a DMA's input
                    # waits block its issuing SEQ, which would stall ScalarE's
                    # next PSUM->SBUF copy if issued from the scalar queue
                    nc.gpsimd.dma_start(quad_ap(out_d, q), as4d(outp[:]))
            psum_a_pool.release()
            psum_x_pool.release()

    nc.compile()
    return nc


def host_prep(x, y, gamma, beta, nr=NR, num_devices=CORES):
    """Shard + encode inputs per core."""
    x = np.ascontiguousarray(np.asarray(x, dtype=np.float32))
    y = np.asarray(y, dtype=np.int32)
    gamma = np.ascontiguousarray(np.asarray(gamma, dtype=np.float32))
    beta = np.ascontiguousarray(np.asarray(beta, dtype=np.float32))
    dom = np.arange(D, dtype=np.int32)
    ident = np.eye(128, dtype=ml_dtypes.bfloat16)
    in_maps = []
    for c in range(num_devices):
        ys = y[c * nr : (c + 1) * nr]
        pairs = nr // 256
        ohw = np.zeros((pairs, 128, MW), dtype=ml_dtypes.bfloat16)
        yp = ys.reshape(pairs, 2, 128)
        ohw[:, :, 0:8] = yp[:, 0, :, None] == dom
        ohw[:, :, 32:40] = yp[:, 1, :, None] == dom
        oh16 = np.ascontiguousarray(ohw.transpose(1, 0, 2).reshape(128, -1))
        oT = np.ascontiguousarray((ys[None, :] == dom[:, None])).astype(
            ml_dtypes.bfloat16
        )
        xs = x[c * nr : (c + 1) * nr]
        xbs = xs.astype(ml_dtypes.bfloat16)
        in_maps.append(
            {
                "xb": xbs,
                "oh16": oh16,
                "oT": oT,
                "ident": ident,
                "gamma": gamma,
                "beta": beta,
            }
        )
    return in_maps


_CACHE = {}


def _get_program():
    if "nc" not in _CACHE:
        _CACHE["nc"] = build_program()
    return _CACHE["nc"]


def kernel(x, y, gamma, beta):
    nc = _get_program()
    in_maps = host_prep(x, y, gamma, beta)
    res = bass_utils.run_bass_kernel_spmd(nc, in_maps, core_ids=list(range(CORES)))
    out = np.empty((N, F), dtype=np.float32)
    for c in range(CORES):
        out[c * NR : (c + 1) * NR] = res.results[c]["out"]
    return out
